# revision 1
# baseline (speedup 1.0000x reference)
"""Trainium2 Bass kernel for nn_Memory_5952824673094.

Reference computes logits = inputs @ mem.T for
inputs [8192, 256] f32, mem [8192, 256] f32 -> out [8192, 8192] f32.

Strategy (8 NeuronCores, data-parallel over batch):
  - Each core gets a 1024-row batch shard of `inputs`; `mem` is replicated.
  - Host passes both operands pre-transposed (contraction dim F=256 on the
    partition axis, split in two 128-chunks) and cast to fp16 — the PE
    accumulates in fp32 PSUM, so precision loss is only the input rounding
    (~2^-11 relative) and this halves input DMA bytes vs f32.
  - Per core: 8 row-tiles x 16 col-tiles of [128, 512] matmuls with K
    accumulated over the 2 chunks in PSUM; PSUM->SBUF copies (cast to fp16)
    split between the Vector and Scalar engines; two 1MB DMas per row-tile
    to DRAM. Output upcast to f32 and concatenated on host.
  - mem loads are chunked (4 column chunks per K-half) and split across the
    two HWDGE queues (SP + ACT) so the PE can start ~5us in; the first two
    row-tiles defer their last-chunk col-tiles so the PE never stalls on
    loads. Steady state is PE-bound at the fp16 streaming floor
    (131072 columns / 2.4GHz = 54.6us per core).

Raw Block-level Bass with manual semaphores (the Tile layer's tail drain
emits multi-wait Drain instructions this toolchain's walrus rejects).
"""

import numpy as np

import concourse.bass as bass
import concourse.mybir as mybir
from contextlib import ExitStack
from concourse.bass_utils import run_bass_kernel_spmd

B, F, C = 8192, 256, 8192
N_CORES = 8
BL = B // N_CORES          # 1024 batch rows per core
P = 128                    # partitions
NB = BL // P               # 8 row (b) tiles per core
CW = 512                   # matmul free-dim / PSUM bank width (f32)
NCT = C // CW              # 16 col (c) tiles
NBANK = 8                  # PSUM banks
N_OT = 3                   # output staging buffers
NCHUNK = 4                 # mem load column chunks per K-half
CHW = C // NCHUNK          # 2048 cols per chunk
IN_DT = mybir.dt.float16
OUT_DT = mybir.dt.float16

# c-tile -> mem chunk
_CHUNK_OF_C = [c * CW // CHW for c in range(NCT)]


def _pass0_order():
    """(t, c) issue order for the first pass: defer the last chunk's c-tiles
    of row-tiles 0 and 1 so the PE never waits on the tail of the mem load."""
    last = NCT - NCT // NCHUNK  # first c-tile of the last chunk (12)
    order = []
    for t in (0, 1):
        order += [(t, c) for c in range(last)]
    for t in (0, 1):
        order += [(t, c) for c in range(last, NCT)]
    for t in range(2, NB):
        order += [(t, c) for c in range(NCT)]
    return order


def _plain_order():
    return [(t, c) for t in range(NB) for c in range(NCT)]


def _copy_engine(c: int) -> str:
    # Even c-tiles on VectorE (DVE), odd on ScalarE (ACT).
    return "v" if c % 2 == 0 else "a"


def build_bass(n_reps: int = 1, timing: bool = False) -> bass.Bass:
    """Build the SPMD program. n_reps>1 repeats the whole pipeline on the
    same data inside one NEFF (for slope-based wall-clock timing: semaphore
    counters simply keep rising across reps, so no resets are needed).
    timing=True keeps the big output in internal DRAM (no device->host
    transfer) and exposes only a tiny dummy output."""
    nc = bass.Bass()
    xT = nc.dram_tensor("xT", [F, BL], IN_DT, kind="ExternalInput")
    memT = nc.dram_tensor("memT", [F, C], IN_DT, kind="ExternalInput")
    if timing:
        out = nc.dram_tensor("out_scratch", [BL, C], OUT_DT)
        dummy = nc.dram_tensor("tiny_out", [P, P], OUT_DT, kind="ExternalOutput")
    else:
        out = nc.dram_tensor("out", [BL, C], OUT_DT, kind="ExternalOutput")
        dummy = None

    # Global instruction schedule: position-ordered (t, c) groups.
    sched = _pass0_order() + _plain_order() * (n_reps - 1)
    n_pos = len(sched)
    # Per-engine copy sequences (positions in engine program order).
    v_pos = [p for p, (t, c) in enumerate(sched) if _copy_engine(c) == "v"]
    a_pos = [p for p, (t, c) in enumerate(sched) if _copy_engine(c) == "a"]
    v_idx = {p: i + 1 for i, p in enumerate(v_pos)}  # position -> 1-based count
    a_idx = {p: i + 1 for i, p in enumerate(a_pos)}

    def copy_wait_args(p):
        """(sem_name, threshold) proving the copy at position p is done."""
        t, c = sched[p]
        if _copy_engine(c) == "v":
            return "v", v_idx[p]
        return "a", a_idx[p]

    # Store schedule: each tile instance tg (pass*NB + t) is stored in 1MB
    # column halves, issued on the SP queue in data-ready order (by the
    # position of the piece's last copy). The first tile's first half and
    # the last tile's last half are split into 0.5MB quarters: the former
    # primes the store queue earlier, the latter shrinks the final-store
    # tail. Thresholds: all copies of that piece (and everything earlier in
    # engine order) are done.
    def _store_entry(tg, c_lo, c_hi):
        r, t = divmod(tg, NB)
        order = _pass0_order() if r == 0 else _plain_order()
        base = 0 if r == 0 else len(_pass0_order()) + (r - 1) * NB * NCT
        positions = [base + order.index((t, c)) for c in range(c_lo, c_hi)]
        thr_v = max((v_idx[p] for p in positions if p in v_idx), default=0)
        thr_a = max((a_idx[p] for p in positions if p in a_idx), default=0)
        return (max(positions), tg, c_lo, c_hi, thr_v, thr_a)

    q = NCT // 4
    pieces = []
    last_tg = n_reps * NB - 1
    for tg in range(n_reps * NB):
        for h in range(2):
            c_lo, c_hi = h * (NCT // 2), (h + 1) * (NCT // 2)
            if (tg == 0 and h == 0) or (tg == last_tg and h == 1):
                pieces.append((tg, c_lo, c_lo + q))
                pieces.append((tg, c_lo + q, c_hi))
            else:
                pieces.append((tg, c_lo, c_hi))
    store_sched = sorted(_store_entry(*pc) for pc in pieces)
    # Per-buffer store counts (cumulative, for the copy-side WAR waits and
    # the final drain): buffer j serves tiles j, j+3, ...
    stores_of_tile = {}
    for tg, c_lo, c_hi in pieces:
        stores_of_tile[tg] = stores_of_tile.get(tg, 0) + 1

    def war_threshold(tg):
        """Sem count proving every store of tiles <= tg - N_OT on tg's
        buffer is complete."""
        return 16 * sum(
            stores_of_tile[t2] for t2 in range(tg % N_OT, tg - N_OT + 1, N_OT)
        )

    with ExitStack() as stk:
        xt = [
            stk.enter_context(nc.sbuf_tensor(f"xt{k}", [P, BL], IN_DT))
            for k in range(2)
        ]
        mt = [
            stk.enter_context(nc.sbuf_tensor(f"mt{k}", [P, C], IN_DT))
            for k in range(2)
        ]
        ot = [
            stk.enter_context(nc.sbuf_tensor(f"ot{k}", [P, C], OUT_DT))
            for k in range(N_OT)
        ]
        ps = [
            stk.enter_context(nc.psum_tensor(f"ps{k}", [P, CW], mybir.dt.float32))
            for k in range(NBANK)
        ]
        # One semaphore per input DMA (completions across HW queues are not
        # ordered); one per staging buffer for output stores (per-buffer
        # stores are serialized by the copy WAR chain).
        s_xt = [stk.enter_context(nc.semaphore(f"s_xt{k}")) for k in range(2)]
        # s_mc[k][j]: mem chunk j of K-half k
        s_mc = [
            [stk.enter_context(nc.semaphore(f"s_mc{k}_{j}")) for j in range(NCHUNK)]
            for k in range(2)
        ]
        s_mm = stk.enter_context(nc.semaphore("s_mm"))
        s_cv = stk.enter_context(nc.semaphore("s_cv"))
        s_ca = stk.enter_context(nc.semaphore("s_ca"))
        s_ob = [stk.enter_context(nc.semaphore(f"s_ob{k}")) for k in range(N_OT)]
        block = stk.enter_context(nc.Block())

        def load_chunk(eng, k, j):
            eng.dma_start(
                out=mt[k][:, j * CHW : (j + 1) * CHW],
                in_=memT[k * P : (k + 1) * P, j * CHW : (j + 1) * CHW],
            ).then_inc(s_mc[k][j], 16)

        n_stores_of = [
            sum(n for tg, n in stores_of_tile.items() if tg % N_OT == j)
            for j in range(N_OT)
        ]

        @block.sync
        def _(sync):
            # SP queue carries only xT, so it is free for the first output
            # store as soon as the first copies land; mem chunks load on the
            # ACT HWDGE queue (0, 2) and the idle GPSIMD SWDGE queue (1, 3).
            sync.dma_start(out=xt[0][:], in_=xT[0:P, :]).then_inc(s_xt[0], 16)
            sync.dma_start(out=xt[1][:], in_=xT[P : 2 * P, :]).then_inc(s_xt[1], 16)
            # Output stores: two column-half DMAs per row-tile, data-ready
            # order.
            for _, tg, c_lo, c_hi, thr_v, thr_a in store_sched:
                t = tg % NB
                sync.wait_ge(s_cv, thr_v)
                sync.wait_ge(s_ca, thr_a)
                cols = slice(c_lo * CW, c_hi * CW)
                sync.dma_start(
                    out=out[t * P : (t + 1) * P, cols],
                    in_=ot[tg % N_OT][:, cols],
                ).then_inc(s_ob[tg % N_OT], 16)
            for j in range(N_OT):
                sync.wait_ge(s_ob[j], 16 * n_stores_of[j])
            if dummy is not None:
                sync.dma_start(out=dummy[:], in_=ot[0][:, 0:P]).then_inc(
                    s_ob[0], 16
                )
                sync.wait_ge(s_ob[0], 16 * (n_stores_of[0] + 1))

        @block.tensor
        def _(tensor):
            tensor.wait_ge(s_xt[0], 16)
            tensor.wait_ge(s_xt[1], 16)
            seen_chunks = set()
            for p, (t, c) in enumerate(sched):
                j = _CHUNK_OF_C[c]
                if j not in seen_chunks:
                    # Only reached during pass 0 (all chunks seen by then).
                    tensor.wait_ge(s_mc[0][j], 16)
                    tensor.wait_ge(s_mc[1][j], 16)
                    seen_chunks.add(j)
                if p >= NBANK:
                    eng, thr = copy_wait_args(p - NBANK)
                    tensor.wait_ge(s_cv if eng == "v" else s_ca, thr)
                bank = ps[p % NBANK]
                tensor.matmul(
                    bank[:],
                    xt[0][:, t * P : (t + 1) * P],
                    mt[0][:, c * CW : (c + 1) * CW],
                    start=True,
                    stop=False,
                )
                tensor.matmul(
                    bank[:],
                    xt[1][:, t * P : (t + 1) * P],
                    mt[1][:, c * CW : (c + 1) * CW],
                    start=False,
                    stop=True,
                ).then_inc(s_mm, 1)

        def copies(eng, positions, sem, is_vector):
            pass0_len = NB * NCT
            last_tg = -1
            for p in positions:
                t, c = sched[p]
                # Tile instance: pass 0 is reordered but stays within tiles
                # 0..NB-1; later passes are plain.
                if p < pass0_len:
                    tg = t
                else:
                    tg = (1 + (p - pass0_len) // (NB * NCT)) * NB + t
                if tg != last_tg and tg >= N_OT:
                    # Staging buffer WAR: all stores of tile tg-N_OT done.
                    eng.wait_ge(s_ob[tg % N_OT], war_threshold(tg))
                last_tg = tg
                eng.wait_ge(s_mm, p + 1)
                dst = ot[tg % N_OT][:, c * CW : (c + 1) * CW]
                if is_vector:
                    eng.tensor_copy(dst, ps[p % NBANK][:]).then_inc(sem, 1)
                else:
                    eng.copy(dst, ps[p % NBANK][:]).then_inc(sem, 1)

        @block.gpsimd
        def _(gpsimd):
            # mem chunks 1 and 3 on the otherwise-idle SWDGE queue.
            for j in (1, 3):
                load_chunk(gpsimd, 0, j)
                load_chunk(gpsimd, 1, j)

        @block.vector
        def _(vector):
            copies(vector, v_pos, s_cv, True)

        @block.scalar
        def _(scalar):
            # mem chunks 0 and 2 load on the ACT HWDGE queue, in parallel
            # with the SP queue's xT + chunks 1, 3.
            for j in (0, 2):
                load_chunk(scalar, 0, j)
                load_chunk(scalar, 1, j)
            copies(scalar, a_pos, s_ca, False)

    return nc


_NC_CACHE = None


def _get_nc() -> bass.Bass:
    global _NC_CACHE
    if _NC_CACHE is None:
        _NC_CACHE = build_bass()
    return _NC_CACHE


def kernel(inputs=None, targets=None, mem=None, epoch=None, **_unused):
    x = np.asarray(inputs, dtype=np.float32)
    m = np.asarray(mem, dtype=np.float32)
    assert x.shape == (B, F) and m.shape == (C, F)

    memT16 = np.ascontiguousarray(m.T).astype(np.float16)
    in_maps = []
    for i in range(N_CORES):
        xs = x[i * BL : (i + 1) * BL]
        in_maps.append(
            {
                "xT": np.ascontiguousarray(xs.T).astype(np.float16),
                "memT": memT16,
            }
        )

    res = run_bass_kernel_spmd(_get_nc(), in_maps, list(range(N_CORES)))
    return np.concatenate(
        [res.results[i]["out"].astype(np.float32) for i in range(N_CORES)], axis=0
    )



# revision 2
# speedup vs baseline: 11.4167x; 11.4167x over previous
"""Trainium2 Bass kernel for nn_Memory_5952824673094.

Reference: logits = inputs @ mem.T for inputs [8192, 256] f32,
mem [8192, 256] f32 -> out [8192, 8192] f32.

Strategy (8 NeuronCores, data-parallel over batch):
  - Each core computes a 1024-row batch shard of the output; `mem` is
    replicated. Operands are host-transposed to fp16 with the
    contraction dim F=256 on the partition axis in two 128-chunks
    (xt[k], mt[k]); the PE accumulates in f32 PSUM, output is stored
    as fp16 and upcast on host (rel err ~3.6e-4).
  - Per core the [1024, 8192] output runs as 16 groups per pass:
    group (t, h) = row-tile t (128 rows) x column half h (8 c-tiles of
    512 cols). Within a group the PE sweeps k-half 0 across all 8 PSUM
    banks, then k-half 1 (accumulating) — the stationary operand
    xt[k][t] is constant across each 8-matmul sweep and walrus's LDW
    dedup pass (--enable-ldw-opt=true, patched in below) folds the 8
    identical LDWEIGHTS into 1. This removes the ~27ns/MM serialized
    weight load that bounds the baseline pattern (measured 72 ->
    66.3 us/iter steady state; the PE streaming floor at the P0
    power-state clock of 2.0 GHz is 65.5 us).
  - h==1 groups run k-half 1 first so the LDW at the (t,0)->(t,1)
    boundary also merges.
  - PSUM bank i -> fp16 staging copy split between DVE (even banks)
    and ACT (odd banks); 1MB output stores per group on the SP queue
    (measured 373 GB/s/core, fully hidden under the PE).
  - mem chunks load once on the ACT HWDGE + GPSIMD SWDGE queues in
    parallel with x on SP; the pass-0 group order defers h=1 groups so
    the PE starts after only the first two chunk loads per queue.

Raw Block-level Bass with manual semaphores. build_bass also provides
timing variants: n_reps>1 repeats the pipeline inside one NEFF for
slope timing; drain=True additionally re-loads all inputs and fully
drains every rep so the slope measures complete single-shot executions
(lead-in + steady + tail) purely on hardware.
"""

import numpy as np

import concourse.bass as bass
import concourse.mybir as mybir
from contextlib import ExitStack
from concourse.bass_utils import run_bass_kernel_spmd

# ---- enable walrus LDWEIGHTS dedup (stock codegen flag, default-off) ----
import concourse.bass_utils as _bu

_orig_run_command = _bu.run_command


def _run_command_ldwopt(argv, **kwargs):
    argv = [
        "--enable-ldw-opt=true" if a == "--enable-ldw-opt=false" else a for a in argv
    ]
    return _orig_run_command(argv, **kwargs)


if getattr(_bu.run_command, "__name__", "") != "_run_command_ldwopt":
    _bu.run_command = _run_command_ldwopt
# ------------------------------------------------------------------------

B, F, C = 8192, 256, 8192
N_CORES = 8
BL = B // N_CORES          # 1024 batch rows per core
P = 128                    # partitions
NB = BL // P               # 8 row (t) tiles per core
CW = 512                   # matmul free-dim / PSUM bank width (f32)
NCT = C // CW              # 16 c-tiles
NBANK = 8                  # PSUM banks
N_OT = 4                   # output staging buffers
NCHUNK = 4                 # mem load column chunks per K-half
CHW = C // NCHUNK          # 2048 cols per chunk
IN_DT = mybir.dt.float16
OUT_DT = mybir.dt.float16


def _pass0_groups():
    """First-pass group order: four h=0 groups first so the PE needs only
    mem chunks 0,1 (first on each load queue) to start."""
    order = [(0, 0), (1, 0), (2, 0), (3, 0), (0, 1), (1, 1), (2, 1), (3, 1)]
    for t in range(4, NB):
        order += [(t, 0), (t, 1)]
    return order


def _plain_groups():
    return [(t, h) for t in range(NB) for h in range(2)]


def build_bass(
    n_reps: int = 1,
    timing: bool = False,
    drain: bool = False,
) -> bass.Bass:
    nc = bass.Bass()
    xT = nc.dram_tensor("xT", [F, BL], IN_DT, kind="ExternalInput")
    memT = nc.dram_tensor("memT", [F, C], IN_DT, kind="ExternalInput")
    if timing:
        out = nc.dram_tensor("out_scratch", [BL, C], OUT_DT)
        dummy = nc.dram_tensor("tiny_out", [P, P], OUT_DT, kind="ExternalOutput")
    else:
        out = nc.dram_tensor("out", [BL, C], OUT_DT, kind="ExternalOutput")
        dummy = None

    if drain:
        groups = _pass0_groups() * n_reps
    else:
        groups = _pass0_groups() + _plain_groups() * (n_reps - 1)
    n_groups = len(groups)

    def tg_of(g):
        return (g // 16) * NB + groups[g][0]

    # Store schedule: one 1MB piece per group on SP in group order after the
    # group's copies; first/last pieces split in half (queue priming /
    # shorter drain tail).
    store_entries = []
    buf_incs = [0] * N_OT
    piece_end = {}
    for g, (t, h) in enumerate(groups):
        tg = tg_of(g)
        buf = tg % N_OT
        base_c = h * (C // 2)
        split = (g % 16 in (0, 15)) if drain else (g in (0, n_groups - 1))
        if split:
            halves = [
                (base_c, base_c + C // 4, 2),
                (base_c + C // 4, base_c + C // 2, 4),
            ]
        else:
            halves = [(base_c, base_c + C // 2, 4)]
        for lo, hi, ncop in halves:
            store_entries.append((g, buf, t, lo, hi, 4 * g + ncop, 4 * g + ncop))
            buf_incs[buf] += 1
        piece_end[(tg, h)] = buf_incs[buf]
    total_buf_incs = list(buf_incs)
    cum_incs = []
    if drain:
        acc = [0] * N_OT
        for r in range(n_reps):
            for e in store_entries:
                if e[0] // 16 == r:
                    acc[e[1]] += 1
            cum_incs.append(list(acc))

    with ExitStack() as stk:
        xt = [
            stk.enter_context(nc.sbuf_tensor(f"xt{k}", [P, BL], IN_DT))
            for k in range(2)
        ]
        mt = [
            stk.enter_context(nc.sbuf_tensor(f"mt{k}", [P, C], IN_DT))
            for k in range(2)
        ]
        ot = [
            stk.enter_context(nc.sbuf_tensor(f"ot{k}", [P, C], OUT_DT))
            for k in range(N_OT)
        ]
        ps = [
            stk.enter_context(nc.psum_tensor(f"ps{k}", [P, CW], mybir.dt.float32))
            for k in range(NBANK)
        ]
        s_xt = [stk.enter_context(nc.semaphore(f"s_xt{k}")) for k in range(2)]
        s_mc = [
            [stk.enter_context(nc.semaphore(f"s_mc{k}_{j}")) for j in range(NCHUNK)]
            for k in range(2)
        ]
        s_mm = stk.enter_context(nc.semaphore("s_mm"))
        s_cv = stk.enter_context(nc.semaphore("s_cv"))
        s_ca = stk.enter_context(nc.semaphore("s_ca"))
        s_ob = [stk.enter_context(nc.semaphore(f"s_ob{k}")) for k in range(N_OT)]
        block = stk.enter_context(nc.Block())

        def load_chunk(eng, k, j):
            eng.dma_start(
                out=mt[k][:, j * CHW : (j + 1) * CHW],
                in_=memT[k * P : (k + 1) * P, j * CHW : (j + 1) * CHW],
            ).then_inc(s_mc[k][j], 16)

        def emit_stores(sync, rep=None):
            for g, buf, t, lo, hi, thr_v, thr_a in store_entries:
                if rep is not None and g // 16 != rep:
                    continue
                sync.wait_ge(s_cv, thr_v)
                sync.wait_ge(s_ca, thr_a)
                sync.dma_start(
                    out=out[t * P : (t + 1) * P, lo:hi],
                    in_=ot[buf][:, lo:hi],
                ).then_inc(s_ob[buf], 16)

        @block.sync
        def _(sync):
            if drain:
                for r in range(n_reps):
                    if r >= 1:
                        for j in range(N_OT):
                            sync.wait_ge(s_ob[j], 16 * cum_incs[r - 1][j])
                    sync.dma_start(out=xt[0][:], in_=xT[0:P, :]).then_inc(
                        s_xt[0], 16
                    )
                    sync.dma_start(out=xt[1][:], in_=xT[P : 2 * P, :]).then_inc(
                        s_xt[1], 16
                    )
                    emit_stores(sync, rep=r)
            else:
                sync.dma_start(out=xt[0][:], in_=xT[0:P, :]).then_inc(s_xt[0], 16)
                sync.dma_start(out=xt[1][:], in_=xT[P : 2 * P, :]).then_inc(
                    s_xt[1], 16
                )
                emit_stores(sync)
            for j in range(N_OT):
                sync.wait_ge(s_ob[j], 16 * total_buf_incs[j])
            if dummy is not None:
                sync.dma_start(out=dummy[:], in_=ot[0][:, 0:P]).then_inc(s_ob[0], 16)
                sync.wait_ge(s_ob[0], 16 * (total_buf_incs[0] + 1))

        @block.tensor
        def _(tensor):
            seen_chunks = set()
            for g, (t, h) in enumerate(groups):
                r = g // 16
                if g == 0 or (drain and g % 16 == 0):
                    if drain and r >= 1:
                        for j in range(N_OT):
                            tensor.wait_ge(s_ob[j], 16 * cum_incs[r - 1][j])
                        seen_chunks = set()
                    tensor.wait_ge(s_xt[0], 16 * (r + 1))
                    tensor.wait_ge(s_xt[1], 16 * (r + 1))
                ks = (0, 1) if h == 0 else (1, 0)
                for j in (2 * h, 2 * h + 1):
                    for k in range(2):
                        if (k, j) not in seen_chunks:
                            tensor.wait_ge(
                                s_mc[k][j], 16 * (r + 1 if drain else 1)
                            )
                            seen_chunks.add((k, j))
                for ki, k in enumerate(ks):
                    stat = xt[k][:, t * P : (t + 1) * P]
                    for i in range(NBANK):
                        c = h * NBANK + i
                        if ki == 0 and g >= 1:
                            if i % 2 == 0:
                                tensor.wait_ge(s_cv, 4 * (g - 1) + i // 2 + 1)
                            else:
                                tensor.wait_ge(s_ca, 4 * (g - 1) + (i - 1) // 2 + 1)
                        mm = tensor.matmul(
                            ps[i][:],
                            stat,
                            mt[k][:, c * CW : (c + 1) * CW],
                            start=(ki == 0),
                            stop=(ki == 1),
                        )
                        if ki == 1:
                            mm.then_inc(s_mm, 1)

        def copies(eng, parity, sem, is_vector, rep=None, barrier=True):
            for g, (t, h) in enumerate(groups):
                if rep is not None and g // 16 != rep:
                    continue
                tg = tg_of(g)
                if drain and barrier and g % 16 == 0 and g >= 16:
                    for j in range(N_OT):
                        eng.wait_ge(s_ob[j], 16 * cum_incs[g // 16 - 1][j])
                if tg >= N_OT:
                    eng.wait_ge(s_ob[tg % N_OT], 16 * piece_end[(tg - N_OT, h)])
                for i in range(parity, NBANK, 2):
                    c = h * NBANK + i
                    eng.wait_ge(s_mm, 8 * g + i + 1)
                    dst = ot[tg % N_OT][:, c * CW : (c + 1) * CW]
                    if is_vector:
                        eng.tensor_copy(dst, ps[i][:]).then_inc(sem, 1)
                    else:
                        eng.copy(dst, ps[i][:]).then_inc(sem, 1)

        @block.gpsimd
        def _(gpsimd):
            if drain:
                for r in range(n_reps):
                    if r >= 1:
                        for j in range(N_OT):
                            gpsimd.wait_ge(s_ob[j], 16 * cum_incs[r - 1][j])
                    for j in (1, 3):
                        load_chunk(gpsimd, 0, j)
                        load_chunk(gpsimd, 1, j)
            else:
                for j in (1, 3):
                    load_chunk(gpsimd, 0, j)
                    load_chunk(gpsimd, 1, j)

        @block.vector
        def _(vector):
            copies(vector, 0, s_cv, True)

        @block.scalar
        def _(scalar):
            if drain:
                for r in range(n_reps):
                    if r >= 1:
                        for j in range(N_OT):
                            scalar.wait_ge(s_ob[j], 16 * cum_incs[r - 1][j])
                    for j in (0, 2):
                        load_chunk(scalar, 0, j)
                        load_chunk(scalar, 1, j)
                    copies(scalar, 1, s_ca, False, rep=r, barrier=False)
            else:
                for j in (0, 2):
                    load_chunk(scalar, 0, j)
                    load_chunk(scalar, 1, j)
                copies(scalar, 1, s_ca, False)

    return nc


_NC_CACHE = None


def _get_nc() -> bass.Bass:
    global _NC_CACHE
    if _NC_CACHE is None:
        _NC_CACHE = build_bass()
    return _NC_CACHE


def kernel(inputs=None, targets=None, mem=None, epoch=None, **_unused):
    x = np.asarray(inputs, dtype=np.float32)
    m = np.asarray(mem, dtype=np.float32)
    assert x.shape == (B, F) and m.shape == (C, F)

    memT16 = np.ascontiguousarray(m.T).astype(np.float16)
    in_maps = []
    for i in range(N_CORES):
        xs = x[i * BL : (i + 1) * BL]
        in_maps.append(
            {
                "xT": np.ascontiguousarray(xs.T).astype(np.float16),
                "memT": memT16,
            }
        )

    res = run_bass_kernel_spmd(_get_nc(), in_maps, list(range(N_CORES)))
    return np.concatenate(
        [res.results[i]["out"].astype(np.float32) for i in range(N_CORES)], axis=0
    )
